# revision 18
# baseline (speedup 1.0000x reference)
import sys

sys.path.insert(0, "/opt/trn_rl_repo")

import numpy as np
import ml_dtypes

import concourse.mybir as mybir
from concourse import bass, tile
from concourse import tile_sem_assignment as _tsa
from concourse.bass_utils import run_bass_kernel_spmd
from concourse.vector_clock import ScopedClock, VectorClock

_orig_drain_and_barrier = tile.TileContext._drain_and_barrier


def _split_drain_and_barrier(self, tick_clock, wait_clock):
    # The final Drain waits on every active semaphore at once; with 8 HWDGE
    # lanes + SWDGE + 3 engines that exceeds the CTRL instruction's sync
    # wait slots. Emit one 1-wait drain per proc instead (same semantics:
    # SP executes them in order, so all sems reach their targets before the
    # barrier), then replicate the original barrier/cleanup sequence.
    gc = tick_clock.global_clock
    n = _tsa.N_PROCS
    for p in range(n):
        if gc[p] > 0:
            partial = VectorClock([gc[q] if q == p else 0 for q in range(n)])
            d = self.nc.sync.drain()
            wait_clock.add_sem_waits(d.ins, ScopedClock({None: partial}))
    self.nc.all_engine_barrier()
    popped = self.nc._tile_sem_poison_stack.pop()
    assert popped is self._sem_poison
    self.nc.clear_and_free_semaphores(list(self.sems.allocated().values()))
    self.nc.all_engine_barrier()


tile.TileContext._drain_and_barrier = _split_drain_and_barrier

B = 1024        # batch rows of address
N = 65536       # mem rows (sharded)
M = 128         # mem cols
NCORES = 8
NS = N // NCORES          # 8192 rows per core
NCHUNKS = NS // 128       # 64 chunks of 128 mem-rows
MCHUNKS = NS // 256       # 32 mega-chunks of 256 mem-rows (DoubleRow)
BCHUNKS = B // 128        # 8 chunks of 128 batch-rows
NSTAGES = 8               # DMA pipeline stages (8 chunks each)

FP8 = mybir.dt.float8e4
BF16 = mybir.dt.bfloat16
F32 = mybir.dt.float32
DR = mybir.MatmulPerfMode.DoubleRow
ADD = mybir.AluOpType.add
MULT = mybir.AluOpType.mult

_compiled = {}


NG = 16  # a DMA groups; each covers 4 mem-chunks (k) = 2 mega-chunks (ch)


def _build_nc():
    nc = bass.Bass(target_bir_lowering=False)

    # a:  [p=b%128, j(n-slice of 1024), ub(u-block), bc, u]  A shard for GEMM1
    a = nc.dram_tensor("a", [128, NSTAGES, 8, BCHUNKS, 128], FP8, kind="ExternalInput")
    # at: [p=n%128 within 256-chunk, ch, sub, b]  A^T shard for GEMM2 (partition=n)
    at = nc.dram_tensor("at", [128, MCHUNKS, 2, B], FP8, kind="ExternalInput")
    # c:  [p=n%128, k, m]  0.5*content shard (partition=n)
    c = nc.dram_tensor("c", [128, NCHUNKS, M], FP8, kind="ExternalInput")
    # ed: [p=b%128, bc, 2M]  [-erase | 0.5*add] fp8 (hi only)
    ed = nc.dram_tensor("ed", [128, BCHUNKS, 2 * M], FP8, kind="ExternalInput")
    # rt: [m, b] partial (read/2)^T bf16
    rt = nc.dram_tensor("rt", [M, B], BF16, kind="ExternalOutput")

    with tile.TileContext(nc) as tc:
        with (
            tc.tile_pool(name="abuf", bufs=1) as a_pool,
            tc.tile_pool(name="atbuf", bufs=1) as at_pool,
            tc.tile_pool(name="cbuf", bufs=1) as c_pool,
            tc.tile_pool(name="edbuf", bufs=1) as ed_pool,
            tc.tile_pool(name="tmpbuf", bufs=8) as tmp_pool,
            tc.tile_pool(name="cpbuf", bufs=6) as cp_pool,
            tc.tile_pool(name="rtbuf", bufs=2) as rt_pool,
            tc.tile_pool(name="pw", bufs=2, space="PSUM") as pw_pool,
            tc.tile_pool(name="pr", bufs=1, space="PSUM") as pr_pool,
        ):
            a_t = a_pool.tile([128, NSTAGES, 8, BCHUNKS, 128], FP8)
            at_t = at_pool.tile([128, MCHUNKS, 2, B], FP8)
            c_t = c_pool.tile([128, NCHUNKS, M], FP8)
            ed_t = ed_pool.tile([128, BCHUNKS, 2 * M], FP8)

            # Transfers issued from different queues run concurrently in the
            # model (the engine SEQ is the serial resource, ~1.58us per
            # 512KB DMA), so spread the input DMAs across all three DMA
            # queues (SP + Act HWDGE, Pool SWDGE) round-robin in global
            # consumption order: DVE eats one n-chunk per ~310ns and needs
            # the matching a-group and at-pair at the same cadence, with a
            # c quarter every 16 chunks. Preload DMAs write each SBUF dest
            # exactly once, so their only wait is the DGE lane-credit wait.
            pieces = []

            def a_group(g):
                j, ub0 = g // 2, (g % 2) * 4
                pieces.append((a_t[:, j, ub0 : ub0 + 4], a[:, j, ub0 : ub0 + 4]))

            def at_pair(p):
                pieces.append((at_t[:, 2 * p : 2 * p + 2], at[:, 2 * p : 2 * p + 2]))

            def c_quarter(qi):
                pieces.append(
                    (c_t[:, 16 * qi : 16 * qi + 16, :], c[:, 16 * qi : 16 * qi + 16, :])
                )

            pieces.append((ed_t[:], ed[:]))
            a_group(0)
            c_quarter(0)
            at_pair(0)
            for g in range(1, 16):
                a_group(g)
                if g == 3:
                    c_quarter(1)
                if g == 7:
                    c_quarter(2)
                if g == 11:
                    c_quarter(3)
                if g < 15:
                    at_pair(g)
            pieces.append((at_t[:, 30:31], at[:, 30:31]))
            pieces.append((at_t[:, 31:32, :, 0:512], at[:, 31:32, :, 0:512]))
            pieces.append((at_t[:, 31:32, :, 512:1024], at[:, 31:32, :, 512:1024]))

            # Pool's first op: memset the dummy-matmul source so the PE
            # p-state warm-up (below) can start immediately.
            wsrcm = tmp_pool.tile([128, 512], FP8)
            nc.gpsimd.memset(wsrcm[:], 0.0)

            queues = [nc.sync, nc.scalar, nc.gpsimd]
            for i, (dst, srcp) in enumerate(pieces):
                queues[i % 3].dma_start(out=dst, in_=srcp)

            # Ramp the PE p-state before the first real G1: the model runs
            # the Tensor engine at 1.2GHz until it has been continuously
            # busy for 3us. Dummy matmuls into psum_r0 (discarded by the
            # first real G2's start=True) span the window until a0 lands.
            psum_r0 = pr_pool.tile([128, 512], F32)
            psum_r1 = pr_pool.tile([128, 512], F32)
            psum_r = [psum_r0, psum_r1]
            for _ in range(8):
                nc.tensor.matmul(
                    psum_r0[0:1, :], wsrcm[:, 0:1], wsrcm[:], start=True, stop=True
                )

            land = tmp_pool.tile([128, 1], F32)
            # Wake the Activation engine early: its first instruction carries
            # a ~1.3us act-table load in the model; pay it off the critical
            # path so the tail copies run at steady-state rate. Source tile
            # is memset by Pool (idle) to avoid waiting on any DMA.
            wsrc = tmp_pool.tile([128, 1], F32)
            warm = tmp_pool.tile([128, 1], F32)
            nc.gpsimd.memset(wsrc[:], 0.0)
            nc.scalar.copy(warm[:], wsrc[:])

            def emit_g2(ch, cp):
                for jj in range(2):
                    nc.tensor.matmul(
                        psum_r[jj][:],
                        cp[:],
                        at_t[:, ch, :, jj * 512 : (jj + 1) * 512],
                        start=(ch == 0),
                        stop=(ch == MCHUNKS - 1),
                        perf_mode=DR,
                    )

            # Process 6 n-chunks per iteration (last group 4): the G1s land
            # in one 3-bank psum tile, and the update runs as ONE fused
            # STT/TT pair over [128, w, M] (strided psum AP), amortizing the
            # per-instruction DVE overhead across 6 chunks.
            for k0 in range(0, NCHUNKS, 6):
                w = min(6, NCHUNKS - k0)
                # DVE absorbs the c DMA wait for this group's last chunk so
                # STT(k0) keeps only its PSUM-read wait (later c-lane waits
                # dedup against the DVE clock).
                nc.vector.tensor_copy(land[:], c_t[:, k0 + w - 1, 0:1])

                cp = cp_pool.tile([128, 6, M], FP8)
                psum_w = pw_pool.tile([128, 6, 2 * M], F32)
                for dk in range(w):
                    k = k0 + dk
                    j, ub = k // 8, k % 8
                    for q in range(4):
                        nc.tensor.matmul(
                            psum_w[:, dk, :],
                            a_t[:, j, ub, 2 * q : 2 * q + 2, :],
                            ed_t[:, 2 * q : 2 * q + 2, :],
                            start=(q == 0),
                            stop=(q == 3),
                            perf_mode=DR,
                        )

                # psum_w[dk] = [-We | Wa/2];  C'/2 = (1 - We)*(C/2) + Wa/2
                tmp2 = tmp_pool.tile([128, 6, M], F32)
                nc.vector.scalar_tensor_tensor(
                    tmp2[:, 0:w, :],
                    psum_w[:, 0:w, 0:M],
                    1.0,
                    c_t[:, k0 : k0 + w, :],
                    ADD,
                    MULT,
                )
                nc.vector.tensor_add(
                    cp[:, 0:w, :], tmp2[:, 0:w, :], psum_w[:, 0:w, M : 2 * M]
                )

                for dch in range(w // 2):
                    emit_g2(k0 // 2 + dch, cp[:, 2 * dch : 2 * dch + 2, :])

            # Split the tail: psum_r bank jj completes at G2(ch=31, jj), so
            # copy+store each half as soon as its accumulation stops. jj0
            # copies on DVE (idle by now) in parallel with jj1 on Act
            # (warmed); stores spread over Pool and SP so neither queues
            # behind the other's issue overhead.
            rt_t0 = rt_pool.tile([128, 512], BF16)
            rt_t1 = rt_pool.tile([128, 512], BF16)
            nc.vector.tensor_copy(rt_t0[:], psum_r0[:])
            s0 = nc.gpsimd.dma_start(out=rt[:, 0:512], in_=rt_t0[:])
            nc.scalar.copy(rt_t1[:], psum_r1[:])
            s1 = nc.sync.dma_start(out=rt[:, 512:1024], in_=rt_t1[:])
            store_names = {s0.ins.name, s1.ins.name}

    # The scheduler can hoist a G1 start-Matmult ahead of the G2 Ldweights
    # whose DVE wait would dedup-cover its bank-WAR wait, leaving it with
    # two waits (PE self-wait + DVE) — one over the HW wait-slot limit.
    # The same-engine self-wait is always satisfied by in-order queue
    # completion, so drop it.
    # The rt stores' RAW wait (on the tail Act copy) transitively follows
    # every input DMA completing, so a DMA-lane credit wait on them is
    # always already satisfied — drop it to stay within the 1-wait limit.
    for inst in nc.inst_map.values():
        si = inst.sync_info
        if si and si.on_wait and len(si.on_wait) > 1:
            eng = str(inst.engine).split(".")[-1]
            kept = [w for w in si.on_wait if not w.ant_name.startswith(eng + "_")]
            if len(kept) > 1 and inst.name in store_names:
                kept = [w for w in kept if not w.ant_name.startswith("DMA")]
            assert len(kept) == 1, (inst.name, [w.ant_name for w in si.on_wait])
            si.on_wait = kept

    return nc


def _prep_inputs(address, erase, add, content):
    f8 = ml_dtypes.float8_e4m3
    a_f8 = address.astype(f8)                                 # [1024, 65536]
    ed = np.concatenate([-erase, 0.5 * add], axis=1)          # [1024, 256] f32
    ed_r = np.ascontiguousarray(
        ed.astype(f8).reshape(BCHUNKS, 128, 2 * M).transpose(1, 0, 2)
    )                                                         # [128, 8, 256]
    c_f8 = (0.5 * content).astype(f8)                         # [65536, 128]

    in_maps = []
    for ci in range(NCORES):
        a_c = a_f8[:, ci * NS : (ci + 1) * NS]                # [1024, 8192]
        # a_r[p, j, ub, bc, u] = A[bc*128+p, j*1024+ub*128+u]
        a_r = np.ascontiguousarray(
            a_c.reshape(BCHUNKS, 128, NSTAGES, 8, 128).transpose(1, 2, 3, 0, 4)
        )                                                     # [128, 8, 8, 8, 128]
        # at_r[p, ch, s, b] = A[b, ch*256 + s*128 + p]
        at_r = np.ascontiguousarray(
            a_c.T.reshape(MCHUNKS, 2, 128, B).transpose(2, 0, 1, 3)
        )                                                     # [128, 32, 2, 1024]
        c_c = c_f8[ci * NS : (ci + 1) * NS, :]
        c_r = np.ascontiguousarray(
            c_c.reshape(NCHUNKS, 128, M).transpose(1, 0, 2)
        )                                                     # [128, 64, 128]
        in_maps.append({"a": a_r, "at": at_r, "c": c_r, "ed": ed_r})
    return in_maps


def kernel(address, erase, add, content, _trace=False, _result_box=None):
    if "nc" not in _compiled:
        _compiled["nc"] = _build_nc()
    nc = _compiled["nc"]

    in_maps = _prep_inputs(address, erase, add, content)
    res = run_bass_kernel_spmd(
        nc, in_maps, core_ids=list(range(NCORES)), trace=_trace
    )
    if _result_box is not None:
        _result_box.append(res)

    acc = np.zeros((M, B), dtype=np.float32)
    for r in res.results:
        acc += np.asarray(r["rt"], dtype=np.float32)
    return np.ascontiguousarray((2.0 * acc).T)


# revision 19
# speedup vs baseline: 1.0241x; 1.0241x over previous
import sys

sys.path.insert(0, "/opt/trn_rl_repo")

import numpy as np
import ml_dtypes

import concourse.mybir as mybir
from concourse import bass, tile
from concourse import tile_sem_assignment as _tsa
from concourse.bass_utils import run_bass_kernel_spmd
from concourse.vector_clock import ScopedClock, VectorClock

_orig_drain_and_barrier = tile.TileContext._drain_and_barrier


def _split_drain_and_barrier(self, tick_clock, wait_clock):
    # The final Drain waits on every active semaphore at once; with 8 HWDGE
    # lanes + SWDGE + 3 engines that exceeds the CTRL instruction's sync
    # wait slots. Emit one 1-wait drain per proc instead (same semantics:
    # SP executes them in order, so all sems reach their targets before the
    # barrier), then replicate the original barrier/cleanup sequence.
    gc = tick_clock.global_clock
    n = _tsa.N_PROCS
    for p in range(n):
        if gc[p] > 0:
            partial = VectorClock([gc[q] if q == p else 0 for q in range(n)])
            d = self.nc.sync.drain()
            wait_clock.add_sem_waits(d.ins, ScopedClock({None: partial}))
    self.nc.all_engine_barrier()
    popped = self.nc._tile_sem_poison_stack.pop()
    assert popped is self._sem_poison
    self.nc.clear_and_free_semaphores(list(self.sems.allocated().values()))
    self.nc.all_engine_barrier()


tile.TileContext._drain_and_barrier = _split_drain_and_barrier

B = 1024        # batch rows of address
N = 65536       # mem rows (sharded)
M = 128         # mem cols
NCORES = 8
NS = N // NCORES          # 8192 rows per core
NCHUNKS = NS // 128       # 64 chunks of 128 mem-rows
MCHUNKS = NS // 256       # 32 mega-chunks of 256 mem-rows (DoubleRow)
BCHUNKS = B // 128        # 8 chunks of 128 batch-rows
NSTAGES = 8               # DMA pipeline stages (8 chunks each)

FP8 = mybir.dt.float8e4
BF16 = mybir.dt.bfloat16
F32 = mybir.dt.float32
DR = mybir.MatmulPerfMode.DoubleRow
ADD = mybir.AluOpType.add
MULT = mybir.AluOpType.mult

_compiled = {}


NG = 16  # a DMA groups; each covers 4 mem-chunks (k) = 2 mega-chunks (ch)


def _build_nc():
    nc = bass.Bass(target_bir_lowering=False)

    # a:  [p=b%128, j(n-slice of 1024), ub(u-block), bc, u]  A shard for GEMM1
    a = nc.dram_tensor("a", [128, NSTAGES, 8, BCHUNKS, 128], FP8, kind="ExternalInput")
    # at: [p=n%128 within 256-chunk, ch, sub, b]  A^T shard for GEMM2 (partition=n)
    at = nc.dram_tensor("at", [128, MCHUNKS, 2, B], FP8, kind="ExternalInput")
    # c:  [p=n%128, k, m]  0.5*content shard (partition=n)
    c = nc.dram_tensor("c", [128, NCHUNKS, M], FP8, kind="ExternalInput")
    # ed: [p=b%128, bc, 2M]  [-erase | 0.5*add] fp8 (hi only)
    ed = nc.dram_tensor("ed", [128, BCHUNKS, 2 * M], FP8, kind="ExternalInput")
    # rt: [m, b] partial (read/2)^T bf16
    rt = nc.dram_tensor("rt", [M, B], BF16, kind="ExternalOutput")

    with tile.TileContext(nc) as tc:
        with (
            tc.tile_pool(name="abuf", bufs=1) as a_pool,
            tc.tile_pool(name="atbuf", bufs=1) as at_pool,
            tc.tile_pool(name="cbuf", bufs=1) as c_pool,
            tc.tile_pool(name="edbuf", bufs=1) as ed_pool,
            tc.tile_pool(name="tmpbuf", bufs=8) as tmp_pool,
            tc.tile_pool(name="cpbuf", bufs=6) as cp_pool,
            tc.tile_pool(name="rtbuf", bufs=2) as rt_pool,
            tc.tile_pool(name="pw", bufs=2, space="PSUM") as pw_pool,
            tc.tile_pool(name="pr", bufs=1, space="PSUM") as pr_pool,
        ):
            a_t = a_pool.tile([128, NSTAGES, 8, BCHUNKS, 128], FP8)
            at_t = at_pool.tile([128, MCHUNKS, 2, B], FP8)
            c_t = c_pool.tile([128, NCHUNKS, M], FP8)
            ed_t = ed_pool.tile([128, BCHUNKS, 2 * M], FP8)

            # Transfers issued from different queues run concurrently in the
            # model (the engine SEQ is the serial resource, ~1.58us per
            # 512KB DMA), so spread the input DMAs across all three DMA
            # queues (SP + Act HWDGE, Pool SWDGE) round-robin in global
            # consumption order: DVE eats one n-chunk per ~310ns and needs
            # the matching a-group and at-pair at the same cadence, with a
            # c quarter every 16 chunks. Preload DMAs write each SBUF dest
            # exactly once, so their only wait is the DGE lane-credit wait.
            pieces = []

            def a_group(g):
                j, ub0 = g // 2, (g % 2) * 4
                pieces.append((a_t[:, j, ub0 : ub0 + 4], a[:, j, ub0 : ub0 + 4]))

            def at_pair(p):
                pieces.append((at_t[:, 2 * p : 2 * p + 2], at[:, 2 * p : 2 * p + 2]))

            def c_quarter(qi):
                pieces.append(
                    (c_t[:, 16 * qi : 16 * qi + 16, :], c[:, 16 * qi : 16 * qi + 16, :])
                )

            pieces.append((ed_t[:], ed[:]))
            a_group(0)
            c_quarter(0)
            a_group(1)
            a_group(2)
            at_pair(0)
            a_group(3)
            at_pair(1)
            c_quarter(1)
            for g in range(4, 16):
                a_group(g)
                if g == 7:
                    c_quarter(2)
                if g == 11:
                    c_quarter(3)
                at_pair(g - 2)
            at_pair(14)
            pieces.append((at_t[:, 30:31], at[:, 30:31]))
            pieces.append((at_t[:, 31:32, :, 0:512], at[:, 31:32, :, 0:512]))
            pieces.append((at_t[:, 31:32, :, 512:1024], at[:, 31:32, :, 512:1024]))

            # Pool's first op: memset the dummy-matmul source so the PE
            # p-state warm-up (below) can start immediately.
            wsrcm = tmp_pool.tile([128, 512], FP8)
            nc.gpsimd.memset(wsrcm[:], 0.0)

            queues = [nc.sync, nc.scalar, nc.gpsimd]
            for i, (dst, srcp) in enumerate(pieces):
                queues[i % 3].dma_start(out=dst, in_=srcp)

            # Ramp the PE p-state before the first real G1: the model runs
            # the Tensor engine at 1.2GHz until it has been continuously
            # busy for 3us. Dummy matmuls into psum_r0 (discarded by the
            # first real G2's start=True) span the window until a0 lands.
            psum_r0 = pr_pool.tile([128, 512], F32)
            psum_r1 = pr_pool.tile([128, 512], F32)
            psum_r = [psum_r0, psum_r1]
            for _ in range(8):
                nc.tensor.matmul(
                    psum_r0[0:1, :], wsrcm[:, 0:1], wsrcm[:], start=True, stop=True
                )

            land = tmp_pool.tile([128, 1], F32)
            # Wake the Activation engine early: its first instruction carries
            # a ~1.3us act-table load in the model; pay it off the critical
            # path so the tail copies run at steady-state rate. Source tile
            # is memset by Pool (idle) to avoid waiting on any DMA.
            wsrc = tmp_pool.tile([128, 1], F32)
            warm = tmp_pool.tile([128, 1], F32)
            nc.gpsimd.memset(wsrc[:], 0.0)
            nc.scalar.copy(warm[:], wsrc[:])

            def emit_g2(ch, cp):
                for jj in range(2):
                    nc.tensor.matmul(
                        psum_r[jj][:],
                        cp[:],
                        at_t[:, ch, :, jj * 512 : (jj + 1) * 512],
                        start=(ch == 0),
                        stop=(ch == MCHUNKS - 1),
                        perf_mode=DR,
                    )

            # Process 6 n-chunks per iteration (last group 4): the G1s land
            # in one 3-bank psum tile, and the update runs as ONE fused
            # STT/TT pair over [128, w, M] (strided psum AP), amortizing the
            # per-instruction DVE overhead across 6 chunks.
            for k0 in [0, 4, 10, 16, 22, 28, 34, 40, 46, 52, 58]:
                w = 4 if k0 == 0 else 6
                # DVE absorbs the c DMA wait for this group's last chunk so
                # STT(k0) keeps only its PSUM-read wait (later c-lane waits
                # dedup against the DVE clock).
                nc.vector.tensor_copy(land[:], c_t[:, k0 + w - 1, 0:1])

                cp = cp_pool.tile([128, 6, M], FP8)
                psum_w = pw_pool.tile([128, 6, 2 * M], F32)
                for dk in range(w):
                    k = k0 + dk
                    j, ub = k // 8, k % 8
                    for q in range(4):
                        nc.tensor.matmul(
                            psum_w[:, dk, :],
                            a_t[:, j, ub, 2 * q : 2 * q + 2, :],
                            ed_t[:, 2 * q : 2 * q + 2, :],
                            start=(q == 0),
                            stop=(q == 3),
                            perf_mode=DR,
                        )

                # psum_w[dk] = [-We | Wa/2];  C'/2 = (1 - We)*(C/2) + Wa/2
                tmp2 = tmp_pool.tile([128, 6, M], F32)
                nc.vector.scalar_tensor_tensor(
                    tmp2[:, 0:w, :],
                    psum_w[:, 0:w, 0:M],
                    1.0,
                    c_t[:, k0 : k0 + w, :],
                    ADD,
                    MULT,
                )
                nc.vector.tensor_add(
                    cp[:, 0:w, :], tmp2[:, 0:w, :], psum_w[:, 0:w, M : 2 * M]
                )

                for dch in range(w // 2):
                    emit_g2(k0 // 2 + dch, cp[:, 2 * dch : 2 * dch + 2, :])

            # Split the tail: psum_r bank jj completes at G2(ch=31, jj), so
            # copy+store each half as soon as its accumulation stops. jj0
            # copies on DVE (idle by now) in parallel with jj1 on Act
            # (warmed); stores spread over Pool and SP so neither queues
            # behind the other's issue overhead.
            rt_t0 = rt_pool.tile([128, 512], BF16)
            rt_t1 = rt_pool.tile([128, 512], BF16)
            nc.vector.tensor_copy(rt_t0[:], psum_r0[:])
            s0 = nc.gpsimd.dma_start(out=rt[:, 0:512], in_=rt_t0[:])
            nc.scalar.copy(rt_t1[:], psum_r1[:])
            s1 = nc.sync.dma_start(out=rt[:, 512:1024], in_=rt_t1[:])
            store_names = {s0.ins.name, s1.ins.name}

    # The scheduler can hoist a G1 start-Matmult ahead of the G2 Ldweights
    # whose DVE wait would dedup-cover its bank-WAR wait, leaving it with
    # two waits (PE self-wait + DVE) — one over the HW wait-slot limit.
    # The same-engine self-wait is always satisfied by in-order queue
    # completion, so drop it.
    # The rt stores' RAW wait (on the tail Act copy) transitively follows
    # every input DMA completing, so a DMA-lane credit wait on them is
    # always already satisfied — drop it to stay within the 1-wait limit.
    for inst in nc.inst_map.values():
        si = inst.sync_info
        if si and si.on_wait and len(si.on_wait) > 1:
            eng = str(inst.engine).split(".")[-1]
            kept = [w for w in si.on_wait if not w.ant_name.startswith(eng + "_")]
            if len(kept) > 1 and inst.name in store_names:
                kept = [w for w in kept if not w.ant_name.startswith("DMA")]
            assert len(kept) == 1, (inst.name, [w.ant_name for w in si.on_wait])
            si.on_wait = kept

    return nc


def _prep_inputs(address, erase, add, content):
    f8 = ml_dtypes.float8_e4m3
    a_f8 = address.astype(f8)                                 # [1024, 65536]
    ed = np.concatenate([-erase, 0.5 * add], axis=1)          # [1024, 256] f32
    ed_r = np.ascontiguousarray(
        ed.astype(f8).reshape(BCHUNKS, 128, 2 * M).transpose(1, 0, 2)
    )                                                         # [128, 8, 256]
    c_f8 = (0.5 * content).astype(f8)                         # [65536, 128]

    in_maps = []
    for ci in range(NCORES):
        a_c = a_f8[:, ci * NS : (ci + 1) * NS]                # [1024, 8192]
        # a_r[p, j, ub, bc, u] = A[bc*128+p, j*1024+ub*128+u]
        a_r = np.ascontiguousarray(
            a_c.reshape(BCHUNKS, 128, NSTAGES, 8, 128).transpose(1, 2, 3, 0, 4)
        )                                                     # [128, 8, 8, 8, 128]
        # at_r[p, ch, s, b] = A[b, ch*256 + s*128 + p]
        at_r = np.ascontiguousarray(
            a_c.T.reshape(MCHUNKS, 2, 128, B).transpose(2, 0, 1, 3)
        )                                                     # [128, 32, 2, 1024]
        c_c = c_f8[ci * NS : (ci + 1) * NS, :]
        c_r = np.ascontiguousarray(
            c_c.reshape(NCHUNKS, 128, M).transpose(1, 0, 2)
        )                                                     # [128, 64, 128]
        in_maps.append({"a": a_r, "at": at_r, "c": c_r, "ed": ed_r})
    return in_maps


def kernel(address, erase, add, content, _trace=False, _result_box=None):
    if "nc" not in _compiled:
        _compiled["nc"] = _build_nc()
    nc = _compiled["nc"]

    in_maps = _prep_inputs(address, erase, add, content)
    res = run_bass_kernel_spmd(
        nc, in_maps, core_ids=list(range(NCORES)), trace=_trace
    )
    if _result_box is not None:
        _result_box.append(res)

    acc = np.zeros((M, B), dtype=np.float32)
    for r in res.results:
        acc += np.asarray(r["rt"], dtype=np.float32)
    return np.ascontiguousarray((2.0 * acc).T)
